# revision 9
# baseline (speedup 1.0000x reference)
"""BinaryNet MLP forward (dense_mlp) on 8 Trainium2 NeuronCores.

Network (reference): x[:, :768] -> binarize -> 4x BinarizeLinear with
BatchNorm(training stats over full batch) + hardtanh + binarize between
layers, log_softmax at the end.

Strategy
--------
Data-parallel over batch: 2048 rows per core; weights replicated
(sign-binarized host-side, shipped as fp8 per the sharding hint's
"1-bit packed" replication idea).

Key ideas (v2, beyond the HBM-round-trip baseline):
  * Matmuls in fp8 DoubleRow (K=256/instruction) -- the PE ceiling here.
  * binarize(hardtanh(batchnorm(h))) == (h >= T) with
    T = mu - b*sqrt(var+eps)/g (sign(g) factor for g<0); for the graded
    g=1,b=0 case T is exactly the batch mean.
  * Pre-binarize h' (= h_true/2, integers in [-2048, 2048]) is stored in
    SBUF as fp8e4: the comparison h' >= T' only flips vs exact when T'
    lands within the fp8 rounding zone of some integer, and |T'| < 8
    while fp8 is integer-exact through 16 -- so fp8 storage is exact for
    the decision.  BN stats stay exact because ScalarE's accum_out sums
    the pre-cast fp32 activation pipe output.
  * Ping-pong activation buffers (A/B) instead of in-place update: the
    layer's binarize writes the *other* buffer, so it can overlap the
    same layer's remaining matmuls instead of waiting for all of them
    (the in-place WAR hazard was the baseline's layer-boundary stall).
  * Per-chunk (4 feature-tiles) batch-stats AllReduce: 8 small ARs per
    layer issued as soon as each chunk's PSUM evictions finish, so only
    the last chunk's AR (+4 binarize ops) is exposed at the boundary.
  * Layer 4 computed transposed (w4 stationary, batch moving): 64
    DoubleRow matmuls instead of 512 LDWEIGHTS-bound tiny matmuls; BN +
    PE-transpose back to batch-major, then the usual log_softmax.
"""

import numpy as np

# Problem sizes (hardcoded per contract).
B = 16384
N_CORES = 8
BC = B // N_CORES          # 2048 rows per core
IND = 768                  # layer-1 contraction (first 768 of 784 cols)
HID = 4096
NOUT = 10
EPS = 1e-5

P = 128                    # SBUF partitions
N_TILE = 512               # matmul moving free dim (one PSUM bank of fp32)
M_PER_CHUNK = 4            # m-tiles (128 feats) per streamed weight panel


def build_program(n_cores=N_CORES, bc=BC, ind=IND, hid=HID, nout=NOUT,
                  enable_asserts=False, general_gamma=False,
                  general_beta=False, shared_ar=False):
    """Build + compile the (SPMD, per-core) Bass program.

    Input DRAM tensors (per core):
      xT   [ind, bc]   bf16   transposed x shard (sign-exact cast)
      w1P/w2P/w3P      fp8e4  sign(w).T pre-arranged in panel order
                       [n_chunks, P, kp*2*MPC*P] so one m-chunk's weights
                       load with a single contiguous DMA
      w4P  [P, kp4*2*nout] fp8e4  sign(w4).T panel (stationary layout)
      g4r/b4r [nout, 1] f32
      (general path) g{l}r/b{l}r [P, mt] f32, feature f=128*m+p at [p,m]
    Output: out [bc, nout] f32
    """
    import concourse.bass as bass
    import concourse.bacc as bacc
    import concourse.tile as tile
    from concourse import mybir
    from concourse.masks import make_identity

    f32 = mybir.dt.float32
    bf16 = mybir.dt.bfloat16
    f8 = mybir.dt.float8e4
    ALU = mybir.AluOpType
    ACTF = mybir.ActivationFunctionType
    DR = mybir.MatmulPerfMode.DoubleRow

    kt1 = ind // P            # k-tiles layer 1 (6)
    kt = hid // P             # k-tiles layers 2,3 (32)
    mt = hid // P             # m-tiles per layer output (32)
    nb = bc // N_TILE         # batch n-tiles of 512 (4)
    nbt = bc // P             # batch tiles of 128 (16)
    n_chunks = mt // M_PER_CHUNK
    kp4 = kt // 2
    nst = 2 if general_beta else 1
    rg = [list(range(n_cores))]
    inv_b = 1.0 / (bc * n_cores)

    nc = bacc.Bacc("TRN2", target_bir_lowering=False, debug=False,
                   enable_asserts=enable_asserts, num_devices=n_cores)

    xT = nc.dram_tensor("xT", [ind, bc], bf16, kind="ExternalInput").ap()
    w1P = nc.dram_tensor("w1P", [n_chunks, P, (ind // P) * M_PER_CHUNK * P],
                         f8, kind="ExternalInput").ap()
    w2P = nc.dram_tensor("w2P", [n_chunks, P, (hid // P) * M_PER_CHUNK * P],
                         f8, kind="ExternalInput").ap()
    w3P = nc.dram_tensor("w3P", [n_chunks, P, (hid // P) * M_PER_CHUNK * P],
                         f8, kind="ExternalInput").ap()
    w4P = nc.dram_tensor("w4P", [P, kp4 * 2 * 16], f8,
                         kind="ExternalInput").ap()
    gb = {}
    if general_gamma or general_beta:
        for l in (1, 2, 3):
            gb[l] = (
                nc.dram_tensor(f"g{l}r", [P, mt], f32,
                               kind="ExternalInput").ap(),
                nc.dram_tensor(f"b{l}r", [P, mt], f32,
                               kind="ExternalInput").ap(),
            )
    g4r = nc.dram_tensor("g4r", [nout, 1], f32, kind="ExternalInput").ap()
    b4r = nc.dram_tensor("b4r", [nout, 1], f32, kind="ExternalInput").ap()
    out_d = nc.dram_tensor("out", [bc, nout], f32, kind="ExternalOutput").ap()

    with tile.TileContext(nc) as tc:
        import contextlib
        with contextlib.ExitStack() as ctx:
            # --- pools ---
            p_acts = ctx.enter_context(tc.tile_pool(name="acts", bufs=1))
            p_xs = ctx.enter_context(tc.tile_pool(name="xs", bufs=3))
            p_wpan = ctx.enter_context(tc.tile_pool(name="wpan", bufs=2))
            p_stat = ctx.enter_context(tc.tile_pool(name="stat", bufs=4))
            p_small = ctx.enter_context(tc.tile_pool(name="small", bufs=1))
            p_psum = ctx.enter_context(
                tc.tile_pool(name="psum", bufs=4, space="PSUM"))
            p_ps4 = ctx.enter_context(
                tc.tile_pool(name="ps4", bufs=2, space="PSUM"))
            p_pm4 = ctx.enter_context(
                tc.tile_pool(name="pm4", bufs=2, space="PSUM"))
            p_dram_ar = ctx.enter_context(
                tc.tile_pool(name="dram_ar", bufs=4, space="DRAM"))
            p_t05 = None
            p_sq = None
            if general_gamma:
                p_t05 = ctx.enter_context(tc.tile_pool(name="t05", bufs=4))
            if general_beta:
                p_sq = ctx.enter_context(tc.tile_pool(name="sqscr", bufs=4))

            # Persistent activation buffers, +-0.5 fp8, feature-major:
            # buf[p, t, b] = activation of feature 128*t+p, batch col b.
            actsX = p_acts.tile([P, kt1, bc], f8)   # layer-1 input
            actsA = p_acts.tile([P, mt, bc], f8)    # L1 out / L3 out
            actsB = p_acts.tile([P, mt, bc], f8)    # L2 out

            # --- prefetch layer-1 weight panels for chunks 0,1 so the
            # first matmul isn't stuck behind the x-shard DMAs ---
            pan_pre = {}
            for c in range(2):
                pan = p_wpan.tile([P, kt1 // 2, 2, M_PER_CHUNK * P], f8,
                                  name=f"pan_l1p{c}", tag="pan")
                nc.sync.dma_start(pan[:], w1P[c])
                pan_pre[c] = pan

            # --- layer 1 input: actsX = sign(x) scaled to +-0.5 ---
            hb = bc // 2
            for t in range(kt1):
                for hh in range(2):
                    xs = p_xs.tile([P, hb], bf16, name="xs")
                    nc.sync.dma_start(
                        xs[:], xT[t * P:(t + 1) * P, hh * hb:(hh + 1) * hb])
                    nc.vector.tensor_scalar(
                        actsX[:, t, hh * hb:(hh + 1) * hb], xs[:], 0.0, 0.5,
                        ALU.is_ge, ALU.subtract)

            def binary_layer(lname, wP, k_tiles, ai, ao, g_ap=None,
                             b_ap=None, pre=None):
                """One BinarizeLinear + BN-threshold layer.

                Reads ai[:, :k_tiles, :]; writes ao[:, :mt, :]: first the
                raw pre-activations h' as fp8 (PSUM eviction with exact
                fp32 stats side-accumulation), then in-place binarize to
                the next layer's +-0.5 activations once the per-chunk
                batch stats have been all-reduced.
                """
                kp = k_tiles // 2
                statp = p_small.tile([P, mt, nst, nb], f32,
                                     name=f"statp_{lname}")
                thr = p_small.tile([P, mt], f32, name=f"thr_{lname}")
                if general_gamma:
                    gl = p_small.tile([P, mt], f32, name=f"g_{lname}")
                    sg = p_small.tile([P, mt], f32, name=f"sg_{lname}")
                    nc.sync.dma_start(gl[:], g_ap[:, :])
                    nc.vector.tensor_scalar(sg[:], gl[:], 0.0, 0.5,
                                            ALU.is_ge, ALU.subtract)
                    nc.vector.tensor_scalar_mul(sg[:], sg[:], 2.0)
                if general_beta:
                    bl = p_small.tile([P, mt], f32, name=f"b_{lname}")
                    nc.sync.dma_start(bl[:], b_ap[:, :])
                    if not general_gamma:
                        gl = p_small.tile([P, mt], f32, name=f"g_{lname}")
                        nc.sync.dma_start(gl[:], g_ap[:, :])

                for c in range(n_chunks):
                    if pre is not None and c in pre:
                        pan = pre[c]
                    else:
                        pan = p_wpan.tile([P, kp, 2, M_PER_CHUNK * P], f8,
                                          name=f"pan_{lname}", tag="pan")
                        nc.sync.dma_start(pan[:], wP[c])
                    for ml in range(M_PER_CHUNK):
                        m = c * M_PER_CHUNK + ml
                        for n in range(nb):
                            ps = p_psum.tile([P, N_TILE], f32, name="ps",
                                             tag="ps")
                            for T in range(kp):
                                nc.tensor.matmul(
                                    ps[:],
                                    pan[:, T, :, ml * P:(ml + 1) * P],
                                    ai[:, 2 * T:2 * T + 2,
                                       n * N_TILE:(n + 1) * N_TILE],
                                    start=(T == 0), stop=(T == kp - 1),
                                    perf_mode=DR)
                            # evict h' to fp8 in the out-acts buffer; the
                            # accumulator keeps the exact fp32 row-sum
                            nsl = slice(n * N_TILE, (n + 1) * N_TILE)
                            nc.scalar.activation(
                                ao[:, m, nsl], ps[:], ACTF.Identity,
                                accum_out=statp[:, m, 0, n:n + 1])
                            if general_beta:
                                sq = p_sq.tile([P, N_TILE], f32, name="sq",
                                               tag="sq")
                                nc.scalar.activation(
                                    sq[:], ps[:], ACTF.Square,
                                    accum_out=statp[:, m, 1, n:n + 1])

                    # ---- per-chunk stats -> AllReduce -> binarize ----
                    # AllReduce the raw per-n-slot sums (64B/partition --
                    # the AR is latency-bound) so nothing upstream of the
                    # AR waits on the DVE queue; ARs then pipeline freely
                    # behind each chunk's evictions.
                    msl = slice(c * M_PER_CHUNK, (c + 1) * M_PER_CHUNK)
                    ar_in = p_dram_ar.tile(
                        [P, M_PER_CHUNK * nst * nb], f32,
                        name=f"ari_{lname}{c}", tag="ari")
                    ar_out = p_dram_ar.tile(
                        [P, M_PER_CHUNK * nst * nb], f32,
                        name=f"aro_{lname}{c}", tag="aro",
                        addr_space="Shared" if shared_ar else "Local")
                    nc.sync.dma_start(ar_in[:], statp[:, msl])
                    nc.gpsimd.collective_compute(
                        "AllReduce", ALU.add, replica_groups=rg,
                        ins=[ar_in.opt()], outs=[ar_out.opt()])
                    statr = p_stat.tile([P, M_PER_CHUNK, nst, nb], f32,
                                        name=f"sr_{lname}{c}", tag="statr")
                    nc.sync.dma_start(statr[:], ar_out[:])

                    # threshold T = mu - b*sqrt(var+eps)/g  (DVE, so the
                    # dependent binarize sits right behind it in-queue)
                    statg = p_stat.tile([P, M_PER_CHUNK, nst], f32,
                                        name=f"sg_{lname}{c}", tag="statg")
                    nc.vector.tensor_reduce(statg[:], statr[:],
                                            mybir.AxisListType.X, ALU.add)
                    if not general_beta:
                        nc.vector.tensor_scalar_mul(thr[:, msl],
                                                    statg[:, :, 0], inv_b)
                    else:
                        mu = p_stat.tile([P, M_PER_CHUNK], f32,
                                         name=f"mu_{lname}{c}", tag="mu")
                        va = p_stat.tile([P, M_PER_CHUNK], f32,
                                         name=f"va_{lname}{c}", tag="va")
                        t2 = p_stat.tile([P, M_PER_CHUNK], f32,
                                         name=f"t2_{lname}{c}", tag="t2")
                        nc.vector.tensor_scalar_mul(mu[:], statg[:, :, 0],
                                                    inv_b)
                        nc.vector.tensor_scalar_mul(va[:], statg[:, :, 1],
                                                    inv_b)
                        nc.vector.tensor_mul(t2[:], mu[:], mu[:])
                        nc.vector.tensor_sub(va[:], va[:], t2[:])
                        # h' stats -> true-scale var is 4x; eps under the
                        # sqrt in true units then back to h' units (/2)
                        nc.vector.tensor_scalar(va[:], va[:], 4.0,
                                                EPS, ALU.mult, ALU.add)
                        nc.scalar.activation(va[:], va[:], ACTF.Sqrt)
                        nc.vector.reciprocal(t2[:], gl[:, msl])
                        nc.vector.tensor_mul(t2[:], t2[:], bl[:, msl])
                        nc.vector.tensor_mul(t2[:], t2[:], va[:])
                        nc.vector.tensor_scalar_mul(t2[:], t2[:], 0.5)
                        nc.vector.tensor_sub(thr[:, msl], mu[:], t2[:])

                    for ml in range(M_PER_CHUNK):
                        m = c * M_PER_CHUNK + ml
                        if general_gamma:
                            t05 = p_t05.tile([P, bc], f8, name="t05",
                                             tag="t05")
                            nc.vector.tensor_scalar(t05[:], ao[:, m, :],
                                                    thr[:, m:m + 1], 0.5,
                                                    ALU.is_ge, ALU.subtract)
                            nc.vector.tensor_scalar(ao[:, m, :], t05[:],
                                                    sg[:, m:m + 1], None,
                                                    ALU.mult)
                        else:
                            nc.vector.tensor_scalar(ao[:, m, :], ao[:, m, :],
                                                    thr[:, m:m + 1], 0.5,
                                                    ALU.is_ge, ALU.subtract)

            binary_layer("l1", w1P, kt1, actsX, actsA,
                         *(gb.get(1) or (None, None)), pre=pan_pre)
            binary_layer("l2", w2P, kt, actsA, actsB,
                         *(gb.get(2) or (None, None)))
            binary_layer("l3", w3P, kt, actsB, actsA,
                         *(gb.get(3) or (None, None)))

            # ---- layer 4, transposed: h4'[o, b] = sign(w4) @ acts ----
            w4sb = p_small.tile([P, kp4, 2, 16], f8)
            nc.sync.dma_start(
                w4sb[:], w4P.rearrange("p (t i f) -> p t i f", t=kp4, i=2))
            g4s = p_small.tile([nout, 1], f32)
            b4s = p_small.tile([nout, 1], f32)
            nc.sync.dma_start(g4s[:], g4r[:, :])
            nc.sync.dma_start(b4s[:], b4r[:, :])

            h4 = p_small.tile([nout, bc], f32)
            s4p = p_small.tile([nout, 2, nb], f32)   # [sum | sumsq] slots
            sq4 = p_small.tile([nout, N_TILE], f32)
            for n in range(nb):
                ps4 = p_ps4.tile([16, N_TILE], f32, name="ps4", tag="ps4")
                for T in range(kp4):
                    nc.tensor.matmul(
                        ps4[:], w4sb[:, T],
                        actsA[:, 2 * T:2 * T + 2,
                              n * N_TILE:(n + 1) * N_TILE],
                        start=(T == 0), stop=(T == kp4 - 1), perf_mode=DR)
                nsl = slice(n * N_TILE, (n + 1) * N_TILE)
                nc.scalar.activation(h4[:, nsl], ps4[0:nout, :], ACTF.Identity,
                                     accum_out=s4p[:, 0, n:n + 1])
                nc.scalar.activation(sq4[:], ps4[0:nout, :], ACTF.Square,
                                     accum_out=s4p[:, 1, n:n + 1])

            ar4_in = p_dram_ar.tile([nout, 2 * nb], f32, name="ar4i",
                                    tag="ari")
            ar4_out = p_dram_ar.tile(
                [nout, 2 * nb], f32, name="ar4o", tag="aro",
                addr_space="Shared" if shared_ar else "Local")
            nc.sync.dma_start(ar4_in[:], s4p[:])
            nc.gpsimd.collective_compute(
                "AllReduce", ALU.add, replica_groups=rg,
                ins=[ar4_in.opt()], outs=[ar4_out.opt()])
            s4r = p_small.tile([nout, 2, nb], f32)
            nc.sync.dma_start(s4r[:], ar4_out[:])
            st4r = p_small.tile([nout, 2], f32)
            nc.vector.tensor_reduce(st4r[:], s4r[:], mybir.AxisListType.X,
                                    ALU.add)

            # BN affine in h' units: y = h4'*A + C
            #   mu' = S1/B, var' = S2/B - mu'^2 (h' units)
            #   rs = 1/sqrt(4*var'+eps); A = 2*g*rs; C = b - 2*mu'*g*rs
            mu4 = p_small.tile([nout, 1], f32)
            va4 = p_small.tile([nout, 1], f32)
            t4 = p_small.tile([nout, 1], f32)
            a4 = p_small.tile([nout, 1], f32)
            c4 = p_small.tile([nout, 1], f32)
            nc.vector.tensor_scalar_mul(mu4[:], st4r[:, 0:1], inv_b)
            nc.vector.tensor_scalar_mul(va4[:], st4r[:, 1:2], inv_b)
            nc.vector.tensor_mul(t4[:], mu4[:], mu4[:])
            nc.vector.tensor_sub(va4[:], va4[:], t4[:])
            nc.vector.tensor_scalar(va4[:], va4[:], 4.0, EPS,
                                    ALU.mult, ALU.add)
            nc.scalar.activation(va4[:], va4[:], ACTF.Sqrt)
            nc.vector.reciprocal(t4[:], va4[:])            # rs
            nc.vector.tensor_mul(t4[:], t4[:], g4s[:])     # g*rs
            nc.vector.tensor_scalar_mul(a4[:], t4[:], 2.0)
            nc.vector.tensor_mul(t4[:], t4[:], mu4[:])
            nc.vector.tensor_scalar_mul(t4[:], t4[:], 2.0)
            nc.vector.tensor_sub(c4[:], b4s[:], t4[:])
            # y = h4*A + C, in place on h4
            nc.vector.tensor_scalar(h4[:], h4[:], a4[:, 0:1], c4[:, 0:1],
                                    ALU.mult, ALU.add)

            # PE-transpose y back to batch-major [P, nbt, nout]
            ident = p_small.tile([nout, nout], f32)
            make_identity(nc, ident[:])
            yT = p_small.tile([P, nbt, nout], f32)
            for bt in range(nbt):
                pst = p_pm4.tile([P, nout], f32, name="pst", tag="pst")
                nc.tensor.transpose(pst[:], h4[:, bt * P:(bt + 1) * P],
                                    ident[:])
                nc.vector.tensor_copy(yT[:, bt, :], pst[:])

            # log_softmax rows
            mx = p_small.tile([P, nbt], f32)
            nc.vector.tensor_reduce(mx[:], yT[:], mybir.AxisListType.X,
                                    ALU.max)
            zt = p_small.tile([P, nbt, nout], f32)
            nc.vector.tensor_tensor(zt[:], yT[:],
                                    mx.broadcast_to([P, nbt, nout]),
                                    ALU.subtract)
            et = p_small.tile([P, nbt, nout], f32)
            nc.scalar.activation(et[:], zt[:], ACTF.Exp)
            se = p_small.tile([P, nbt], f32)
            nc.vector.tensor_reduce(se[:], et[:], mybir.AxisListType.X,
                                    ALU.add)
            lse = p_small.tile([P, nbt], f32)
            nc.scalar.activation(lse[:], se[:], ACTF.Ln)
            ot = p_small.tile([P, nbt, nout], f32)
            nc.vector.tensor_tensor(ot[:], zt[:],
                                    lse.broadcast_to([P, nbt, nout]),
                                    ALU.subtract)
            nc.sync.dma_start(out_d.rearrange("(t p) f -> p t f", p=P),
                              ot[:])

    nc.compile()
    return nc


_CACHE = {}


def _get_program(general_gamma=False, general_beta=False):
    key = ("nc", general_gamma, general_beta)
    if key not in _CACHE:
        _CACHE[key] = build_program(general_gamma=general_gamma,
                                    general_beta=general_beta)
    return _CACHE[key]


def _prep_shared(w1, w2, w3, w4, g1, b1, g2, b2, g3, b3, g4, b4,
                 general=False):
    import ml_dtypes
    f = np.float32
    f8 = ml_dtypes.float8_e4m3
    kp4 = HID // 256

    def t(a):
        # sign(w).T as fp8 {-1,+1}; >=0 -> +1 exactly as reference binarize
        a = np.asarray(a, dtype=f)
        return np.where(a.T >= 0, np.float32(1.0),
                        np.float32(-1.0)).astype(f8)

    def pan(wT8):
        # [K, F] -> [F//512, P, K*4] panel order: chunk-contiguous weights
        # (c, p, T, i, m) = wT8[256T+128i+p, 512c+m]
        K, F = wT8.shape
        kp, nch = K // 256, F // (M_PER_CHUNK * P)
        v = wT8.reshape(kp, 2, P, nch, M_PER_CHUNK * P)
        return np.ascontiguousarray(
            v.transpose(3, 2, 0, 1, 4)).reshape(nch, P, K * M_PER_CHUNK)

    w4T8 = t(w4)  # [hid, nout]
    w4pad = np.zeros((HID, 16), dtype=w4T8.dtype)
    w4pad[:, :NOUT] = w4T8
    w4p = np.ascontiguousarray(
        w4pad.reshape(kp4, 2, P, 16).transpose(2, 0, 1, 3)
    ).reshape(P, kp4 * 2 * 16)

    out = {
        "w1P": pan(t(w1)), "w2P": pan(t(w2)), "w3P": pan(t(w3)),
        "w4P": w4p,
        "g4r": np.asarray(g4, dtype=f).reshape(NOUT, 1).copy(),
        "b4r": np.asarray(b4, dtype=f).reshape(NOUT, 1).copy(),
    }
    if general:
        def r(v):
            v = np.asarray(v, dtype=f)
            return np.ascontiguousarray(v.reshape(-1, P).T)  # [P, mt]
        out.update({
            "g1r": r(g1), "b1r": r(b1), "g2r": r(g2), "b2r": r(b2),
            "g3r": r(g3), "b3r": r(b3),
        })
    return out


def kernel(x, w1, w2, w3, w4, g1, b1, g2, b2, g3, b3, g4, b4):
    import ml_dtypes
    from concourse.bass_utils import run_bass_kernel_spmd

    gen_g = not all(np.all(np.asarray(g) > 0) for g in (g1, g2, g3))
    gen_b = not all(np.all(np.asarray(b) == 0) for b in (b1, b2, b3))
    nc = _get_program(general_gamma=gen_g, general_beta=gen_b)
    shared = _prep_shared(w1, w2, w3, w4, g1, b1, g2, b2, g3, b3, g4, b4,
                          general=(gen_g or gen_b))
    xs = np.asarray(x, dtype=np.float32).reshape(-1, 784)[:, :IND]
    in_maps = []
    for c in range(N_CORES):
        m = dict(shared)
        m["xT"] = np.ascontiguousarray(
            xs[c * BC:(c + 1) * BC, :].T).astype(ml_dtypes.bfloat16)
        in_maps.append(m)
    res = run_bass_kernel_spmd(nc, in_maps, core_ids=list(range(N_CORES)))
    return np.concatenate([res.results[c]["out"] for c in range(N_CORES)],
                          axis=0)


# revision 10
# speedup vs baseline: 1.2429x; 1.2429x over previous
"""BinaryNet MLP forward (dense_mlp) on 8 Trainium2 NeuronCores.

Network (reference): x[:, :768] -> binarize -> 4x BinarizeLinear with
BatchNorm(training stats over full batch) + hardtanh + binarize between
layers, log_softmax at the end.

Strategy
--------
Data-parallel over batch: 2048 rows per core; weights replicated
(sign-binarized host-side, shipped as fp8 per the sharding hint's
"1-bit packed" replication idea).

Key ideas (v2, beyond the HBM-round-trip baseline):
  * Matmuls in fp8 DoubleRow (K=256/instruction) -- the PE ceiling here.
  * binarize(hardtanh(batchnorm(h))) == (h >= T) with
    T = mu - b*sqrt(var+eps)/g (sign(g) factor for g<0); for the graded
    g=1,b=0 case T is exactly the batch mean.
  * Pre-binarize h' (= h_true/2, integers in [-2048, 2048]) is stored in
    SBUF as fp8e4: the comparison h' >= T' only flips vs exact when T'
    lands within the fp8 rounding zone of some integer, and |T'| < 8
    while fp8 is integer-exact through 16 -- so fp8 storage is exact for
    the decision.  BN stats stay exact because ScalarE's accum_out sums
    the pre-cast fp32 activation pipe output.
  * Ping-pong activation buffers (A/B) instead of in-place update: the
    layer's binarize writes the *other* buffer, so it can overlap the
    same layer's remaining matmuls instead of waiting for all of them
    (the in-place WAR hazard was the baseline's layer-boundary stall).
  * Per-chunk (4 feature-tiles) batch-stats AllReduce: 8 small ARs per
    layer issued as soon as each chunk's PSUM evictions finish, so only
    the last chunk's AR (+4 binarize ops) is exposed at the boundary.
  * Layer 4 computed transposed (w4 stationary, batch moving): 64
    DoubleRow matmuls instead of 512 LDWEIGHTS-bound tiny matmuls; BN +
    PE-transpose back to batch-major, then the usual log_softmax.
"""

import numpy as np

# Problem sizes (hardcoded per contract).
B = 16384
N_CORES = 8
BC = B // N_CORES          # 2048 rows per core
IND = 768                  # layer-1 contraction (first 768 of 784 cols)
HID = 4096
NOUT = 10
EPS = 1e-5

P = 128                    # SBUF partitions
N_TILE = 512               # matmul moving free dim (one PSUM bank of fp32)
M_PER_CHUNK = 4            # m-tiles (128 feats) per streamed weight panel


def build_program(n_cores=N_CORES, bc=BC, ind=IND, hid=HID, nout=NOUT,
                  enable_asserts=False, general_gamma=False,
                  general_beta=False, shared_ar=True):
    """Build + compile the (SPMD, per-core) Bass program.

    Input DRAM tensors (per core):
      xT   [ind, bc]   bf16   transposed x shard (sign-exact cast)
      w1P/w2P/w3P      fp8e4  sign(w).T pre-arranged in panel order
                       [n_chunks, P, kp*2*MPC*P] so one m-chunk's weights
                       load with a single contiguous DMA
      w4P  [P, kp4*2*nout] fp8e4  sign(w4).T panel (stationary layout)
      g4r/b4r [nout, 1] f32
      (general path) g{l}r/b{l}r [P, mt] f32, feature f=128*m+p at [p,m]
    Output: out [bc, nout] f32
    """
    import concourse.bass as bass
    import concourse.bacc as bacc
    import concourse.tile as tile
    from concourse import mybir
    from concourse.masks import make_identity

    f32 = mybir.dt.float32
    bf16 = mybir.dt.bfloat16
    f8 = mybir.dt.float8e4
    ALU = mybir.AluOpType
    ACTF = mybir.ActivationFunctionType
    DR = mybir.MatmulPerfMode.DoubleRow

    kt1 = ind // P            # k-tiles layer 1 (6)
    kt = hid // P             # k-tiles layers 2,3 (32)
    mt = hid // P             # m-tiles per layer output (32)
    nb = bc // N_TILE         # batch n-tiles of 512 (4)
    nbt = bc // P             # batch tiles of 128 (16)
    n_chunks = mt // M_PER_CHUNK
    kp4 = kt // 2
    nst = 2 if general_beta else 1
    rg = [list(range(n_cores))]
    inv_b = 1.0 / (bc * n_cores)

    nc = bacc.Bacc("TRN2", target_bir_lowering=False, debug=False,
                   enable_asserts=enable_asserts, num_devices=n_cores)

    xT = nc.dram_tensor("xT", [ind, bc], bf16, kind="ExternalInput").ap()
    w1P = nc.dram_tensor("w1P", [n_chunks, P, (ind // P) * M_PER_CHUNK * P],
                         f8, kind="ExternalInput").ap()
    w2P = nc.dram_tensor("w2P", [n_chunks, P, (hid // P) * M_PER_CHUNK * P],
                         f8, kind="ExternalInput").ap()
    w3P = nc.dram_tensor("w3P", [n_chunks, P, (hid // P) * M_PER_CHUNK * P],
                         f8, kind="ExternalInput").ap()
    w4P = nc.dram_tensor("w4P", [P, kp4 * 2 * 16], f8,
                         kind="ExternalInput").ap()
    gb = {}
    if general_gamma or general_beta:
        for l in (1, 2, 3):
            gb[l] = (
                nc.dram_tensor(f"g{l}r", [P, mt], f32,
                               kind="ExternalInput").ap(),
                nc.dram_tensor(f"b{l}r", [P, mt], f32,
                               kind="ExternalInput").ap(),
            )
    g4r = nc.dram_tensor("g4r", [nout, 1], f32, kind="ExternalInput").ap()
    b4r = nc.dram_tensor("b4r", [nout, 1], f32, kind="ExternalInput").ap()
    out_d = nc.dram_tensor("out", [bc, nout], f32, kind="ExternalOutput").ap()

    with tile.TileContext(nc) as tc:
        import contextlib
        with contextlib.ExitStack() as ctx:
            # --- pools ---
            p_acts = ctx.enter_context(tc.tile_pool(name="acts", bufs=1))
            p_xs = ctx.enter_context(tc.tile_pool(name="xs", bufs=3))
            p_wpan = ctx.enter_context(tc.tile_pool(name="wpan", bufs=2))
            p_stat = ctx.enter_context(tc.tile_pool(name="stat", bufs=4))
            p_small = ctx.enter_context(tc.tile_pool(name="small", bufs=1))
            p_psum = ctx.enter_context(
                tc.tile_pool(name="psum", bufs=4, space="PSUM"))
            p_ps4 = ctx.enter_context(
                tc.tile_pool(name="ps4", bufs=2, space="PSUM"))
            p_pm4 = ctx.enter_context(
                tc.tile_pool(name="pm4", bufs=2, space="PSUM"))
            p_dram_ar = ctx.enter_context(
                tc.tile_pool(name="dram_ar", bufs=4, space="DRAM"))
            p_t05 = None
            p_sq = None
            if general_gamma:
                p_t05 = ctx.enter_context(tc.tile_pool(name="t05", bufs=4))
            if general_beta:
                p_sq = ctx.enter_context(tc.tile_pool(name="sqscr", bufs=4))

            # Persistent activation buffers, +-0.5 fp8, feature-major:
            # buf[p, t, b] = activation of feature 128*t+p, batch col b.
            actsX = p_acts.tile([P, kt1, bc], f8)   # layer-1 input
            actsA = p_acts.tile([P, mt, bc], f8)    # L1 out / L3 out
            actsB = p_acts.tile([P, mt, bc], f8)    # L2 out

            # --- prefetch layer-1 weight panels for chunks 0,1 so the
            # first matmul isn't stuck behind the x-shard DMAs ---
            pan_pre = {}
            for c in range(2):
                pan = p_wpan.tile([P, kt1 // 2, 2, M_PER_CHUNK * P], f8,
                                  name=f"pan_l1p{c}", tag="pan")
                nc.sync.dma_start(pan[:], w1P[c])
                pan_pre[c] = pan

            # --- layer 1 input: actsX = sign(x) scaled to +-0.5 ---
            hb = bc // 2
            for t in range(kt1):
                for hh in range(2):
                    xs = p_xs.tile([P, hb], bf16, name="xs")
                    nc.sync.dma_start(
                        xs[:], xT[t * P:(t + 1) * P, hh * hb:(hh + 1) * hb])
                    nc.vector.tensor_scalar(
                        actsX[:, t, hh * hb:(hh + 1) * hb], xs[:], 0.0, 0.5,
                        ALU.is_ge, ALU.subtract)

            def binary_layer(lname, wP, k_tiles, ai, ao, g_ap=None,
                             b_ap=None, pre=None):
                """One BinarizeLinear + BN-threshold layer.

                Reads ai[:, :k_tiles, :]; writes ao[:, :mt, :]: first the
                raw pre-activations h' as fp8 (PSUM eviction with exact
                fp32 stats side-accumulation), then in-place binarize to
                the next layer's +-0.5 activations once the per-chunk
                batch stats have been all-reduced.
                """
                kp = k_tiles // 2
                statp = p_small.tile([P, mt, nst, nb], f32,
                                     name=f"statp_{lname}")
                thr = p_small.tile([P, mt], f32, name=f"thr_{lname}")
                if general_gamma:
                    gl = p_small.tile([P, mt], f32, name=f"g_{lname}")
                    sg = p_small.tile([P, mt], f32, name=f"sg_{lname}")
                    nc.sync.dma_start(gl[:], g_ap[:, :])
                    nc.vector.tensor_scalar(sg[:], gl[:], 0.0, 0.5,
                                            ALU.is_ge, ALU.subtract)
                    nc.vector.tensor_scalar_mul(sg[:], sg[:], 2.0)
                if general_beta:
                    bl = p_small.tile([P, mt], f32, name=f"b_{lname}")
                    nc.sync.dma_start(bl[:], b_ap[:, :])
                    if not general_gamma:
                        gl = p_small.tile([P, mt], f32, name=f"g_{lname}")
                        nc.sync.dma_start(gl[:], g_ap[:, :])

                for c in range(n_chunks):
                    if pre is not None and c in pre:
                        pan = pre[c]
                    else:
                        pan = p_wpan.tile([P, kp, 2, M_PER_CHUNK * P], f8,
                                          name=f"pan_{lname}", tag="pan")
                        nc.sync.dma_start(pan[:], wP[c])
                    for ml in range(M_PER_CHUNK):
                        m = c * M_PER_CHUNK + ml
                        for n in range(nb):
                            ps = p_psum.tile([P, N_TILE], f32, name="ps",
                                             tag="ps")
                            for T in range(kp):
                                nc.tensor.matmul(
                                    ps[:],
                                    pan[:, T, :, ml * P:(ml + 1) * P],
                                    ai[:, 2 * T:2 * T + 2,
                                       n * N_TILE:(n + 1) * N_TILE],
                                    start=(T == 0), stop=(T == kp - 1),
                                    perf_mode=DR)
                            # evict h' to fp8 in the out-acts buffer; the
                            # accumulator keeps the exact fp32 row-sum
                            nsl = slice(n * N_TILE, (n + 1) * N_TILE)
                            nc.scalar.activation(
                                ao[:, m, nsl], ps[:], ACTF.Identity,
                                accum_out=statp[:, m, 0, n:n + 1])
                            if general_beta:
                                sq = p_sq.tile([P, N_TILE], f32, name="sq",
                                               tag="sq")
                                nc.scalar.activation(
                                    sq[:], ps[:], ACTF.Square,
                                    accum_out=statp[:, m, 1, n:n + 1])

                    # ---- per-chunk stats -> AllReduce -> binarize ----
                    # AllReduce the raw per-n-slot sums (64B/partition --
                    # the AR is latency-bound) so nothing upstream of the
                    # AR waits on the DVE queue; ARs then pipeline freely
                    # behind each chunk's evictions.
                    msl = slice(c * M_PER_CHUNK, (c + 1) * M_PER_CHUNK)
                    ar_in = p_dram_ar.tile(
                        [P, M_PER_CHUNK * nst * nb], f32,
                        name=f"ari_{lname}{c}", tag="ari")
                    ar_out = p_dram_ar.tile(
                        [P, M_PER_CHUNK * nst * nb], f32,
                        name=f"aro_{lname}{c}", tag="aro",
                        addr_space="Shared" if shared_ar else "Local")
                    nc.sync.dma_start(ar_in[:], statp[:, msl])
                    nc.gpsimd.collective_compute(
                        "AllReduce", ALU.add, replica_groups=rg,
                        ins=[ar_in.opt()], outs=[ar_out.opt()])
                    statr = p_stat.tile([P, M_PER_CHUNK, nst, nb], f32,
                                        name=f"sr_{lname}{c}", tag="statr")
                    nc.sync.dma_start(statr[:], ar_out[:])

                    # threshold T = mu - b*sqrt(var+eps)/g  (DVE, so the
                    # dependent binarize sits right behind it in-queue)
                    statg = p_stat.tile([P, M_PER_CHUNK, nst], f32,
                                        name=f"sg_{lname}{c}", tag="statg")
                    nc.vector.tensor_reduce(statg[:], statr[:],
                                            mybir.AxisListType.X, ALU.add)
                    if not general_beta:
                        nc.vector.tensor_scalar_mul(thr[:, msl],
                                                    statg[:, :, 0], inv_b)
                    else:
                        mu = p_stat.tile([P, M_PER_CHUNK], f32,
                                         name=f"mu_{lname}{c}", tag="mu")
                        va = p_stat.tile([P, M_PER_CHUNK], f32,
                                         name=f"va_{lname}{c}", tag="va")
                        t2 = p_stat.tile([P, M_PER_CHUNK], f32,
                                         name=f"t2_{lname}{c}", tag="t2")
                        nc.vector.tensor_scalar_mul(mu[:], statg[:, :, 0],
                                                    inv_b)
                        nc.vector.tensor_scalar_mul(va[:], statg[:, :, 1],
                                                    inv_b)
                        nc.vector.tensor_mul(t2[:], mu[:], mu[:])
                        nc.vector.tensor_sub(va[:], va[:], t2[:])
                        # h' stats -> true-scale var is 4x; eps under the
                        # sqrt in true units then back to h' units (/2)
                        nc.vector.tensor_scalar(va[:], va[:], 4.0,
                                                EPS, ALU.mult, ALU.add)
                        nc.scalar.activation(va[:], va[:], ACTF.Sqrt)
                        nc.vector.reciprocal(t2[:], gl[:, msl])
                        nc.vector.tensor_mul(t2[:], t2[:], bl[:, msl])
                        nc.vector.tensor_mul(t2[:], t2[:], va[:])
                        nc.vector.tensor_scalar_mul(t2[:], t2[:], 0.5)
                        nc.vector.tensor_sub(thr[:, msl], mu[:], t2[:])

                    for ml in range(M_PER_CHUNK):
                        m = c * M_PER_CHUNK + ml
                        if general_gamma:
                            t05 = p_t05.tile([P, bc], f8, name="t05",
                                             tag="t05")
                            nc.vector.tensor_scalar(t05[:], ao[:, m, :],
                                                    thr[:, m:m + 1], 0.5,
                                                    ALU.is_ge, ALU.subtract)
                            nc.vector.tensor_scalar(ao[:, m, :], t05[:],
                                                    sg[:, m:m + 1], None,
                                                    ALU.mult)
                        else:
                            nc.vector.tensor_scalar(ao[:, m, :], ao[:, m, :],
                                                    thr[:, m:m + 1], 0.5,
                                                    ALU.is_ge, ALU.subtract)

            binary_layer("l1", w1P, kt1, actsX, actsA,
                         *(gb.get(1) or (None, None)), pre=pan_pre)
            binary_layer("l2", w2P, kt, actsA, actsB,
                         *(gb.get(2) or (None, None)))
            binary_layer("l3", w3P, kt, actsB, actsA,
                         *(gb.get(3) or (None, None)))

            # ---- layer 4, transposed: h4'[o, b] = sign(w4) @ acts ----
            w4sb = p_small.tile([P, kp4, 2, 16], f8)
            nc.sync.dma_start(
                w4sb[:], w4P.rearrange("p (t i f) -> p t i f", t=kp4, i=2))
            g4s = p_small.tile([nout, 1], f32)
            b4s = p_small.tile([nout, 1], f32)
            nc.sync.dma_start(g4s[:], g4r[:, :])
            nc.sync.dma_start(b4s[:], b4r[:, :])

            h4 = p_small.tile([nout, bc], f32)
            s4p = p_small.tile([nout, 2, nb], f32)   # [sum | sumsq] slots
            sq4 = p_small.tile([nout, N_TILE], f32)
            for n in range(nb):
                ps4 = p_ps4.tile([16, N_TILE], f32, name="ps4", tag="ps4")
                for T in range(kp4):
                    nc.tensor.matmul(
                        ps4[:], w4sb[:, T],
                        actsA[:, 2 * T:2 * T + 2,
                              n * N_TILE:(n + 1) * N_TILE],
                        start=(T == 0), stop=(T == kp4 - 1), perf_mode=DR)
                nsl = slice(n * N_TILE, (n + 1) * N_TILE)
                nc.scalar.activation(h4[:, nsl], ps4[0:nout, :], ACTF.Identity,
                                     accum_out=s4p[:, 0, n:n + 1])
                nc.scalar.activation(sq4[:], ps4[0:nout, :], ACTF.Square,
                                     accum_out=s4p[:, 1, n:n + 1])

            ar4_in = p_dram_ar.tile([nout, 2 * nb], f32, name="ar4i",
                                    tag="ari")
            ar4_out = p_dram_ar.tile(
                [nout, 2 * nb], f32, name="ar4o", tag="aro",
                addr_space="Shared" if shared_ar else "Local")
            nc.sync.dma_start(ar4_in[:], s4p[:])
            nc.gpsimd.collective_compute(
                "AllReduce", ALU.add, replica_groups=rg,
                ins=[ar4_in.opt()], outs=[ar4_out.opt()])
            s4r = p_small.tile([nout, 2, nb], f32)
            nc.sync.dma_start(s4r[:], ar4_out[:])
            st4r = p_small.tile([nout, 2], f32)
            nc.vector.tensor_reduce(st4r[:], s4r[:], mybir.AxisListType.X,
                                    ALU.add)

            # BN affine in h' units: y = h4'*A + C
            #   mu' = S1/B, var' = S2/B - mu'^2 (h' units)
            #   rs = 1/sqrt(4*var'+eps); A = 2*g*rs; C = b - 2*mu'*g*rs
            mu4 = p_small.tile([nout, 1], f32)
            va4 = p_small.tile([nout, 1], f32)
            t4 = p_small.tile([nout, 1], f32)
            a4 = p_small.tile([nout, 1], f32)
            c4 = p_small.tile([nout, 1], f32)
            nc.vector.tensor_scalar_mul(mu4[:], st4r[:, 0:1], inv_b)
            nc.vector.tensor_scalar_mul(va4[:], st4r[:, 1:2], inv_b)
            nc.vector.tensor_mul(t4[:], mu4[:], mu4[:])
            nc.vector.tensor_sub(va4[:], va4[:], t4[:])
            nc.vector.tensor_scalar(va4[:], va4[:], 4.0, EPS,
                                    ALU.mult, ALU.add)
            nc.scalar.activation(va4[:], va4[:], ACTF.Sqrt)
            nc.vector.reciprocal(t4[:], va4[:])            # rs
            nc.vector.tensor_mul(t4[:], t4[:], g4s[:])     # g*rs
            nc.vector.tensor_scalar_mul(a4[:], t4[:], 2.0)
            nc.vector.tensor_mul(t4[:], t4[:], mu4[:])
            nc.vector.tensor_scalar_mul(t4[:], t4[:], 2.0)
            nc.vector.tensor_sub(c4[:], b4s[:], t4[:])
            # y = h4*A + C, in place on h4
            nc.vector.tensor_scalar(h4[:], h4[:], a4[:, 0:1], c4[:, 0:1],
                                    ALU.mult, ALU.add)

            # PE-transpose y back to batch-major [P, nbt, nout]
            ident = p_small.tile([nout, nout], f32)
            make_identity(nc, ident[:])
            yT = p_small.tile([P, nbt, nout], f32)
            for bt in range(nbt):
                pst = p_pm4.tile([P, nout], f32, name="pst", tag="pst")
                nc.tensor.transpose(pst[:], h4[:, bt * P:(bt + 1) * P],
                                    ident[:])
                nc.vector.tensor_copy(yT[:, bt, :], pst[:])

            # log_softmax rows
            mx = p_small.tile([P, nbt], f32)
            nc.vector.tensor_reduce(mx[:], yT[:], mybir.AxisListType.X,
                                    ALU.max)
            zt = p_small.tile([P, nbt, nout], f32)
            nc.vector.tensor_tensor(zt[:], yT[:],
                                    mx.broadcast_to([P, nbt, nout]),
                                    ALU.subtract)
            et = p_small.tile([P, nbt, nout], f32)
            nc.scalar.activation(et[:], zt[:], ACTF.Exp)
            se = p_small.tile([P, nbt], f32)
            nc.vector.tensor_reduce(se[:], et[:], mybir.AxisListType.X,
                                    ALU.add)
            lse = p_small.tile([P, nbt], f32)
            nc.scalar.activation(lse[:], se[:], ACTF.Ln)
            ot = p_small.tile([P, nbt, nout], f32)
            nc.vector.tensor_tensor(ot[:], zt[:],
                                    lse.broadcast_to([P, nbt, nout]),
                                    ALU.subtract)
            nc.sync.dma_start(out_d.rearrange("(t p) f -> p t f", p=P),
                              ot[:])

    nc.compile()
    return nc


_CACHE = {}


def _get_program(general_gamma=False, general_beta=False):
    key = ("nc", general_gamma, general_beta)
    if key not in _CACHE:
        _CACHE[key] = build_program(general_gamma=general_gamma,
                                    general_beta=general_beta)
    return _CACHE[key]


def _prep_shared(w1, w2, w3, w4, g1, b1, g2, b2, g3, b3, g4, b4,
                 general=False):
    import ml_dtypes
    f = np.float32
    f8 = ml_dtypes.float8_e4m3
    kp4 = HID // 256

    def t(a):
        # sign(w).T as fp8 {-1,+1}; >=0 -> +1 exactly as reference binarize
        a = np.asarray(a, dtype=f)
        return np.where(a.T >= 0, np.float32(1.0),
                        np.float32(-1.0)).astype(f8)

    def pan(wT8):
        # [K, F] -> [F//512, P, K*4] panel order: chunk-contiguous weights
        # (c, p, T, i, m) = wT8[256T+128i+p, 512c+m]
        K, F = wT8.shape
        kp, nch = K // 256, F // (M_PER_CHUNK * P)
        v = wT8.reshape(kp, 2, P, nch, M_PER_CHUNK * P)
        return np.ascontiguousarray(
            v.transpose(3, 2, 0, 1, 4)).reshape(nch, P, K * M_PER_CHUNK)

    w4T8 = t(w4)  # [hid, nout]
    w4pad = np.zeros((HID, 16), dtype=w4T8.dtype)
    w4pad[:, :NOUT] = w4T8
    w4p = np.ascontiguousarray(
        w4pad.reshape(kp4, 2, P, 16).transpose(2, 0, 1, 3)
    ).reshape(P, kp4 * 2 * 16)

    out = {
        "w1P": pan(t(w1)), "w2P": pan(t(w2)), "w3P": pan(t(w3)),
        "w4P": w4p,
        "g4r": np.asarray(g4, dtype=f).reshape(NOUT, 1).copy(),
        "b4r": np.asarray(b4, dtype=f).reshape(NOUT, 1).copy(),
    }
    if general:
        def r(v):
            v = np.asarray(v, dtype=f)
            return np.ascontiguousarray(v.reshape(-1, P).T)  # [P, mt]
        out.update({
            "g1r": r(g1), "b1r": r(b1), "g2r": r(g2), "b2r": r(b2),
            "g3r": r(g3), "b3r": r(b3),
        })
    return out


def kernel(x, w1, w2, w3, w4, g1, b1, g2, b2, g3, b3, g4, b4):
    import ml_dtypes
    from concourse.bass_utils import run_bass_kernel_spmd

    gen_g = not all(np.all(np.asarray(g) > 0) for g in (g1, g2, g3))
    gen_b = not all(np.all(np.asarray(b) == 0) for b in (b1, b2, b3))
    nc = _get_program(general_gamma=gen_g, general_beta=gen_b)
    shared = _prep_shared(w1, w2, w3, w4, g1, b1, g2, b2, g3, b3, g4, b4,
                          general=(gen_g or gen_b))
    xs = np.asarray(x, dtype=np.float32).reshape(-1, 784)[:, :IND]
    in_maps = []
    for c in range(N_CORES):
        m = dict(shared)
        m["xT"] = np.ascontiguousarray(
            xs[c * BC:(c + 1) * BC, :].T).astype(ml_dtypes.bfloat16)
        in_maps.append(m)
    res = run_bass_kernel_spmd(nc, in_maps, core_ids=list(range(N_CORES)))
    return np.concatenate([res.results[c]["out"] for c in range(N_CORES)],
                          axis=0)
